# revision 6
# baseline (speedup 1.0000x reference)
"""Multi-head attention (B=4, S=2048, D=1024, H=16) on 8 Trainium2 cores.

Sharding (v6): core c -> head-pair p = c (2 heads, 128 output dims), all 4
batches.  valid_len truncation is SPMD-uniform: every core runs the same
per-batch k-loop trip counts kc_b = ceil(valid_len[b]/128).  W_o is
row-split by head-pair; each core emits a full-shape [B, S, D] fp16
partial and the host sums the 8 partials.

Masking: the host zeroes xv columns at k >= valid_len[b] and supplies a
masked ones-column, so masked keys contribute exactly 0 to both the
attention*V accumulation and the softmax denominator.

v6 schedule (from the v5 trace: PE 66% busy at the 1.2 GHz mid p-state,
per-iteration stalls waiting on exp, Sync engine 62% busy issuing 434
DMAs, ACT half-loaded with copies):
  - The attention inner loop is software-pipelined: scores(kc+1) is
    emitted BEFORE av(kc), so the PE never sits behind exp(kc) (the Tile
    scheduler honours emission order as priority).
  - Q/K/V and O projections are emitted as small closures interleaved
    into the attention stream ("fillers"), keeping the PE continuously
    busy so it ramps to the 2.4 GHz p-state.
  - Inputs are staged whole-batch in [128, S]-wide SBUF tiles (24 big
    DMAs per batch instead of ~100 small ones), prefetched one stage
    ahead; PSUM->SBUF copies all run on DVE so ACT does exp only.
  - O-projection writes one [128, 1024] fp16 tile per 128 rows -> one
    output DMA per tile.
"""

import contextlib
from collections import deque

import numpy as np
import ml_dtypes

import concourse.bacc as bacc
import concourse.mybir as mybir
import concourse.tile as tile
from concourse.bass_utils import run_bass_kernel_spmd

BF16 = mybir.dt.bfloat16
F16 = mybir.dt.float16
F32 = mybir.dt.float32
AF = mybir.ActivationFunctionType

B, S, D, H, HD = 4, 2048, 1024, 16, 64
NQB = S // 512        # query blocks of 512

_cache = {}


def _emit(nc, tc, ap, kcs):
    es = contextlib.ExitStack()
    with es:
        const = es.enter_context(tc.tile_pool(name="const", bufs=1))
        resid = es.enter_context(tc.tile_pool(name="resid", bufs=1))
        xstage = es.enter_context(tc.tile_pool(name="xstage", bufs=1))
        expool = es.enter_context(tc.tile_pool(name="expool", bufs=4))
        wrk = es.enter_context(tc.tile_pool(name="wrk", bufs=2))
        otp = es.enter_context(tc.tile_pool(name="otp", bufs=4))

        # constants: per-dj [din-chunk, dout=128] weight tiles for the pair
        wq_sb = [const.tile([128, 128], BF16, tag=f"wq{i}", name=f"wq{i}")
                 for i in range(8)]
        wk_sb = [const.tile([128, 128], BF16, tag=f"wk{i}", name=f"wk{i}")
                 for i in range(8)]
        wv_sb = [const.tile([128, 2, HD], BF16, tag=f"wv{i}", name=f"wv{i}")
                 for i in range(8)]
        wo_sb = const.tile([128, D], BF16, tag="wo", name="wo")
        vm_sb = const.tile([128, 64, 2], BF16, tag="vmask", name="vmask")
        nc.sync.dma_start(vm_sb[:], ap["vones"])
        for i in range(8):
            nc.sync.dma_start(wq_sb[i][:], ap["wq"][i * 128:(i + 1) * 128, :])
            nc.sync.dma_start(wk_sb[i][:], ap["wk"][i * 128:(i + 1) * 128, :])
            nc.sync.dma_start(wv_sb[i][:],
                              ap["wv"][i * 128:(i + 1) * 128, :, :])
        nc.sync.dma_start(wo_sb[:], ap["wo"])

        # residents (per batch)
        qT_sb = [resid.tile([128, S], BF16, tag=f"qT{b}", name=f"qT{b}")
                 for b in range(B)]
        kT_sb = [resid.tile([128, kcs[b] * 128], BF16, tag=f"kT{b}",
                            name=f"kT{b}") for b in range(B)]
        ctx_sb = [resid.tile([128, S], BF16, tag=f"ctx{b}", name=f"ctx{b}")
                  for b in range(B)]
        v_sb = [[resid.tile([128, 2, HD + 1], BF16, tag=f"v{b}_{i}",
                            name=f"v{b}_{i}") for i in range(kcs[b])]
                for b in range(B)]

        # whole-batch input staging, one buffer per dj chunk (reused across
        # batches; Tile's WAR deps serialize refill against the last reader)
        xq_st = [xstage.tile([128, S], BF16, tag=f"sxq{i}", name=f"sxq{i}")
                 for i in range(8)]
        xk_st = [xstage.tile([128, S], BF16, tag=f"sxk{i}", name=f"sxk{i}")
                 for i in range(8)]
        xv_st = [xstage.tile([128, S], BF16, tag=f"sxv{i}", name=f"sxv{i}")
                 for i in range(8)]

        def prefetch_kv(b):
            nk = kcs[b] * 128
            for i in range(8):
                nc.gpsimd.dma_start(xk_st[i][:, 0:nk],
                                    ap[f"xk{b}"][i * 128:(i + 1) * 128, :])
            for i in range(8):
                nc.gpsimd.dma_start(xv_st[i][:, 0:nk],
                                    ap[f"xv{b}"][i * 128:(i + 1) * 128, :])

        def prefetch_q(b):
            for i in range(8):
                nc.gpsimd.dma_start(xq_st[i][:],
                                    ap[f"xq{b}"][i * 128:(i + 1) * 128, :])

        def prefetch_s0(b):
            # arrival-ordered 1024-col chunks so stage-0 projections can
            # start as soon as the first column window lands (K, then V on
            # the Pool queue; Q behind the weights on the Sync queue)
            nk = kcs[b] * 128
            for st, apn in ((xk_st, f"xk{b}"), (xv_st, f"xv{b}")):
                cuts = [0, 512, 1024] + list(range(2048, nk + 1, 1024))
                cuts = sorted(set(c for c in cuts if c <= nk) | {nk})
                for n0, n1 in zip(cuts[:-1], cuts[1:]):
                    for i in range(8):
                        nc.gpsimd.dma_start(
                            st[i][:, n0:n1],
                            ap[apn][i * 128:(i + 1) * 128, n0:n1])
            for c in range(2):
                for i in range(8):
                    nc.sync.dma_start(
                        xq_st[i][:, c * 1024:(c + 1) * 1024],
                        ap[f"xq{b}"][i * 128:(i + 1) * 128,
                                     c * 1024:(c + 1) * 1024])

        fill = es.enter_context(
            tc.tile_pool(name="fill_psum", bufs=1, space="PSUM"))
        at_psum = es.enter_context(
            tc.tile_pool(name="at_psum", bufs=1, space="PSUM"))

        # p-state warmup: ~7us of dependency-free matmuls on the already-
        # loaded weight tiles so the tensor engine ramps to full clock
        # while the first input chunks are still in flight
        for w in range(48):
            pw = fill.tile([128, 128], F32,
                           tag=("pqk" if w % 2 == 0 else "pv"), name="pw")
            nc.tensor.matmul(pw[:], wq_sb[w % 8][:], wk_sb[(w + 1) % 8][:],
                             start=True, stop=True)

        state = {"drain": False}

        def ps_copy(dst, src_ap):
            if state["drain"]:
                nc.scalar.activation(dst, src_ap, AF.Copy)
            else:
                nc.vector.tensor_copy(dst, src_ap)

        # ---- filler closures (one PE burst each) ----
        def kproj_block(b, kb):
            nk = kcs[b] * 128
            n = min(512, nk - kb * 512)

            def go():
                psk = fill.tile([128, 512], F32, tag="pqk", name="psk")
                for dj in range(8):
                    nc.tensor.matmul(
                        psk[:, 0:n],
                        wk_sb[dj][:],
                        xk_st[dj][:, kb * 512:kb * 512 + n],
                        start=(dj == 0), stop=(dj == 7))
                ps_copy(kT_sb[b][:, kb * 512:kb * 512 + n], psk[:, 0:n])
            return go

        def vproj_sc(b, sc):
            def go():
                psv = fill.tile([128, 2, HD], F32, tag="pv", name="psv")
                for dj in range(8):
                    nc.tensor.matmul(
                        psv[:], xv_st[dj][:, sc * 128:(sc + 1) * 128],
                        wv_sb[dj][:], start=(dj == 0), stop=(dj == 7))
                nc.vector.tensor_copy(v_sb[b][sc][:, :, 0:HD], psv[:])
                nc.vector.tensor_copy(
                    v_sb[b][sc][:, :, HD], vm_sb[:, b * 16 + sc, :])
            return go

        def qproj_qb(b, qb):
            def go():
                psq = fill.tile([128, 512], F32, tag="pqk", name="psq")
                for dj in range(8):
                    nc.tensor.matmul(
                        psq[:], wq_sb[dj][:],
                        xq_st[dj][:, qb * 512:(qb + 1) * 512],
                        start=(dj == 0), stop=(dj == 7))
                ps_copy(qT_sb[b][:, qb * 512:(qb + 1) * 512], psq[:])
            return go

        def oproj_sc(b, sc):
            def go():
                ot = otp.tile([128, 2, 512], F16, tag="ot", name="ot")
                for ih in range(2):
                    po = fill.tile([128, 512], F32,
                                   tag=("pqk" if ih == 0 else "pv"), name="po")
                    nc.tensor.matmul(
                        po[:], ctx_sb[b][:, sc * 128:(sc + 1) * 128],
                        wo_sb[:, ih * 512:(ih + 1) * 512],
                        start=True, stop=True)
                    if ih == 1:
                        nc.scalar.activation(ot[:, ih, :], po[:], AF.Copy)
                    else:
                        nc.vector.tensor_copy(ot[:, ih, :], po[:])
                nc.sync.dma_start(
                    ap["out"][b, sc * 128:(sc + 1) * 128, 0:512],
                    ot[:, 0, :])
                nc.scalar.dma_start(
                    ap["out"][b, sc * 128:(sc + 1) * 128, 512:1024],
                    ot[:, 1, :])
            return go

        def proj_closures(b):
            cl = []
            nk = kcs[b] * 128
            for kb in range((nk + 511) // 512):
                cl.append(kproj_block(b, kb))
            for sc in range(kcs[b]):
                cl.append(vproj_sc(b, sc))
            for qb in range(NQB):
                cl.append(qproj_qb(b, qb))
            return cl

        # stage order: descending kc, but smallest second-to-last so the
        # final stage has enough attention iterations to host o-proj fillers
        stages = sorted(range(B), key=lambda b: -kcs[b])
        stages = stages[:-2] + [stages[-1], stages[-2]]

        proj_fifo = deque()
        oproj_fifo = deque()

        def pop_filler():
            if proj_fifo:
                proj_fifo.popleft()()
                return True
            if oproj_fifo:
                oproj_fifo.popleft()()
                return True
            return False

        def attn(b):
            """Attention for batch b; fillers run between sc(kc+1)/av(kc)."""
            kcb = kcs[b]
            it = 0
            for qb in range(NQB):
                # o-proj of group qb-2 is safely complete by now
                if qb >= 2:
                    for sc in range(4 * (qb - 2), 4 * (qb - 1)):
                        oproj_fifo.append(oproj_sc(b, sc))
                av = at_psum.tile([HD + 1, 2, 512], F32, tag="av", name="av")

                def sc_mm(kc):
                    scp = at_psum.tile([128, 1024], F32, tag="sc",
                                       name="scp", bufs=2)
                    for h2 in range(2):
                        nc.tensor.matmul(
                            scp[:, h2 * 512:(h2 + 1) * 512],
                            kT_sb[b][64 * h2:64 * h2 + 64,
                                     kc * 128:(kc + 1) * 128],
                            qT_sb[b][64 * h2:64 * h2 + 64,
                                     qb * 512:(qb + 1) * 512],
                            start=True, stop=True)
                    return scp

                scp_cur = sc_mm(0)
                for kc in range(kcb):
                    scp_next = sc_mm(kc + 1) if kc + 1 < kcb else None
                    ex = expool.tile([128, 1024], BF16, tag="ex", name="ex")
                    nc.scalar.activation(ex[:], scp_cur[:], AF.Exp,
                                         scale=0.125)
                    # paced filler between sc(kc+1) and av(kc): it covers
                    # the exp latency and keeps the PE p-state high
                    if it % 2 == 1 or len(oproj_fifo) > 8:
                        pop_filler()
                    it += 1
                    for h2 in range(2):
                        nc.tensor.matmul(
                            av[:, h2, :], v_sb[b][kc][:, h2, :],
                            ex[:, h2 * 512:(h2 + 1) * 512],
                            start=(kc == 0), stop=(kc == kcb - 1))
                    scp_cur = scp_next

                # normalization: ctx[m, q] = av[m, q] / av[64, q]
                avc = wrk.tile([HD + 1, 2, 512], F32, tag="avc", name="avc")
                nc.vector.tensor_copy(avc[:, 0, :], av[:, 0, :])
                nc.scalar.activation(avc[:, 1, :], av[:, 1, :], AF.Copy)
                r0 = wrk.tile([1, 1024], F32, tag="r0", name="r0")
                nc.sync.dma_start(r0[:], avc[HD:HD + 1, :, :])
                bc = wrk.tile([HD, 1024], F32, tag="bc", name="bc")
                nc.gpsimd.partition_broadcast(bc[:], r0[0:1, :])
                recb = wrk.tile([HD, 1024], F32, tag="recb", name="recb")
                nc.vector.reciprocal_approx_fast(recb[:], bc[:])
                nc.vector.tensor_mul(
                    ctx_sb[b][0:HD, qb * 512:(qb + 1) * 512],
                    avc[0:HD, 0, :], recb[:, 0:512])
                tmp = wrk.tile([HD, 512], BF16, tag="tmpb", name="tmp")
                nc.vector.tensor_mul(tmp[:], avc[0:HD, 1, :],
                                     recb[:, 512:1024])
                nc.sync.dma_start(
                    ctx_sb[b][HD:128, qb * 512:(qb + 1) * 512], tmp[:])
            # last two qb groups become available for later stages
            for sc in range(4 * (NQB - 2), 4 * NQB):
                oproj_fifo.append(oproj_sc(b, sc))

        # ---- stage 0: inline projection, then pipelined attention ----
        s0 = stages[0]
        prefetch_s0(s0)
        for cl in proj_closures(s0):
            cl()
        if len(stages) > 1:
            prefetch_kv(stages[1])
            prefetch_q(stages[1])

        for si, b in enumerate(stages):
            if si + 1 < len(stages):
                proj_fifo.extend(proj_closures(stages[si + 1]))
                if si + 2 < len(stages):
                    nb = stages[si + 2]
                    proj_fifo.append(lambda nb=nb: prefetch_kv(nb))
                    proj_fifo.append(lambda nb=nb: prefetch_q(nb))
            attn(b)
            # drain projections for the next stage before its attention;
            # PSUM copies go to ACT here (no exp work competing)
            state["drain"] = True
            while proj_fifo:
                proj_fifo.popleft()()
            state["drain"] = False
        state["drain"] = True
        while oproj_fifo:
            oproj_fifo.popleft()()
        state["drain"] = False


def _build(kcs):
    key = ("nc", tuple(kcs))
    if key in _cache:
        return _cache[key]
    nc = bacc.Bacc("TRN2", target_bir_lowering=False, debug=False, num_devices=8)
    ap = {"wq": nc.dram_tensor("wq", [D, 128], BF16, kind="ExternalInput").ap(),
          "wk": nc.dram_tensor("wk", [D, 128], BF16, kind="ExternalInput").ap(),
          "wv": nc.dram_tensor("wv", [D, 2, HD], BF16, kind="ExternalInput").ap(),
          "wo": nc.dram_tensor("wo", [128, D], BF16, kind="ExternalInput").ap(),
          "vones": nc.dram_tensor("vones", [128, 64, 2], BF16,
                                  kind="ExternalInput").ap(),
          "out": nc.dram_tensor("out", [B, S, D], F16,
                                kind="ExternalOutput").ap()}
    for b in range(B):
        ap[f"xq{b}"] = nc.dram_tensor(f"xq{b}", [D, S], BF16,
                                      kind="ExternalInput").ap()
        ap[f"xk{b}"] = nc.dram_tensor(f"xk{b}", [D, kcs[b] * 128], BF16,
                                      kind="ExternalInput").ap()
        ap[f"xv{b}"] = nc.dram_tensor(f"xv{b}", [D, kcs[b] * 128], BF16,
                                      kind="ExternalInput").ap()
    with tile.TileContext(nc) as tc:
        _emit(nc, tc, ap, kcs)
    nc.compile()
    _cache[key] = nc
    return nc


def _in_maps(kcs, queries, keys, values, valid_len, W_q, W_k, W_v, W_o):
    bf = ml_dtypes.bfloat16
    # host-masked ones column: 1 where k < valid_len[b], else 0
    # vones[p, b*16+sc, h] = 1 if sc*128+p < valid_len[b] else 0
    kpos = np.arange(16 * 128).reshape(16, 128)
    vones = np.zeros((128, 64, 2), bf)
    for b in range(B):
        v1 = (kpos < int(valid_len[b])).astype(bf)  # [16, 128]
        vones[:, b * 16:(b + 1) * 16, :] = v1.T[:, :, None]
    maps = []
    for c in range(8):
        j0 = 128 * c
        m = {
            "wq": np.ascontiguousarray(W_q[j0:j0 + 128, :].T).astype(bf),
            "wk": np.ascontiguousarray(W_k[j0:j0 + 128, :].T).astype(bf),
            "wv": np.ascontiguousarray(
                W_v[j0:j0 + 128, :].T).astype(bf).reshape(D, 2, HD),
            "wo": np.ascontiguousarray(W_o[:, j0:j0 + 128].T).astype(bf),
            "vones": vones,
        }
        for b in range(B):
            nk = kcs[b] * 128
            xv = values[b][:nk].T.copy()      # [D, nk]
            xv[:, int(valid_len[b]):] = 0.0   # mask padding rows of v
            m[f"xq{b}"] = np.ascontiguousarray(queries[b].T).astype(bf)
            m[f"xk{b}"] = np.ascontiguousarray(keys[b][:nk].T).astype(bf)
            m[f"xv{b}"] = xv.astype(bf)
        maps.append(m)
    return maps


def kernel(queries, keys, values, valid_len, W_q, W_k, W_v, W_o, _run_kwargs=None):
    queries = np.asarray(queries, np.float32)
    keys = np.asarray(keys, np.float32)
    values = np.asarray(values, np.float32)
    valid_len = np.asarray(valid_len)
    W_q = np.asarray(W_q, np.float32)
    W_k = np.asarray(W_k, np.float32)
    W_v = np.asarray(W_v, np.float32)
    W_o = np.asarray(W_o, np.float32)

    kcs = [max(1, min(16, -(-int(valid_len[b]) // 128))) for b in range(B)]
    nc = _build(kcs)
    maps = _in_maps(kcs, queries, keys, values, valid_len, W_q, W_k, W_v, W_o)
    res = run_bass_kernel_spmd(nc, maps, list(range(8)), **(_run_kwargs or {}))
    out = np.zeros((B, S, D), np.float32)
    for c in range(8):
        out += res.results[c]["out"].astype(np.float32)
    if _run_kwargs:
        _cache["last_results"] = res
    return out


# revision 7
# speedup vs baseline: 1.0737x; 1.0737x over previous
"""Multi-head attention (B=4, S=2048, D=1024, H=16) on 8 Trainium2 cores.

Sharding (v6): core c -> head-pair p = c (2 heads, 128 output dims), all 4
batches.  valid_len truncation is SPMD-uniform: every core runs the same
per-batch k-loop trip counts kc_b = ceil(valid_len[b]/128).  W_o is
row-split by head-pair; each core emits a full-shape [B, S, D] fp16
partial and the host sums the 8 partials.

Masking: the host zeroes xv columns at k >= valid_len[b] and supplies a
masked ones-column, so masked keys contribute exactly 0 to both the
attention*V accumulation and the softmax denominator.

v6 schedule (from the v5 trace: PE 66% busy at the 1.2 GHz mid p-state,
per-iteration stalls waiting on exp, Sync engine 62% busy issuing 434
DMAs, ACT half-loaded with copies):
  - The attention inner loop is software-pipelined: scores(kc+1) is
    emitted BEFORE av(kc), so the PE never sits behind exp(kc) (the Tile
    scheduler honours emission order as priority).
  - Q/K/V and O projections are emitted as small closures interleaved
    into the attention stream ("fillers"), keeping the PE continuously
    busy so it ramps to the 2.4 GHz p-state.
  - Inputs are staged whole-batch in [128, S]-wide SBUF tiles (24 big
    DMAs per batch instead of ~100 small ones), prefetched one stage
    ahead; PSUM->SBUF copies all run on DVE so ACT does exp only.
  - O-projection writes one [128, 1024] fp16 tile per 128 rows -> one
    output DMA per tile.
"""

import contextlib
from collections import deque

import numpy as np
import ml_dtypes

import concourse.bacc as bacc
import concourse.mybir as mybir
import concourse.tile as tile
from concourse.bass_utils import run_bass_kernel_spmd

BF16 = mybir.dt.bfloat16
F16 = mybir.dt.float16
F32 = mybir.dt.float32
AF = mybir.ActivationFunctionType

B, S, D, H, HD = 4, 2048, 1024, 16, 64
NQB = S // 512        # query blocks of 512

_cache = {}


def _emit(nc, tc, ap, kcs):
    es = contextlib.ExitStack()
    with es:
        const = es.enter_context(tc.tile_pool(name="const", bufs=1))
        resid = es.enter_context(tc.tile_pool(name="resid", bufs=1))
        xstage = es.enter_context(tc.tile_pool(name="xstage", bufs=1))
        expool = es.enter_context(tc.tile_pool(name="expool", bufs=4))
        wrk = es.enter_context(tc.tile_pool(name="wrk", bufs=2))
        otp = es.enter_context(tc.tile_pool(name="otp", bufs=4))

        # constants: per-dj [din-chunk, dout=128] weight tiles for the pair
        wq_sb = [const.tile([128, 128], BF16, tag=f"wq{i}", name=f"wq{i}")
                 for i in range(8)]
        wk_sb = [const.tile([128, 128], BF16, tag=f"wk{i}", name=f"wk{i}")
                 for i in range(8)]
        wv_sb = [const.tile([128, 2, HD], BF16, tag=f"wv{i}", name=f"wv{i}")
                 for i in range(8)]
        wo_sb = const.tile([128, D], BF16, tag="wo", name="wo")
        vm_sb = const.tile([128, 64, 2], BF16, tag="vmask", name="vmask")
        nc.sync.dma_start(vm_sb[:], ap["vones"])
        for i in range(8):
            nc.sync.dma_start(wq_sb[i][:], ap["wq"][i * 128:(i + 1) * 128, :])
            nc.sync.dma_start(wk_sb[i][:], ap["wk"][i * 128:(i + 1) * 128, :])
            nc.sync.dma_start(wv_sb[i][:],
                              ap["wv"][i * 128:(i + 1) * 128, :, :])
        nc.sync.dma_start(wo_sb[:], ap["wo"])

        # residents (per batch)
        qT_sb = [resid.tile([128, S], BF16, tag=f"qT{b}", name=f"qT{b}")
                 for b in range(B)]
        kT_sb = [resid.tile([128, kcs[b] * 128], BF16, tag=f"kT{b}",
                            name=f"kT{b}") for b in range(B)]
        ctx_sb = [resid.tile([128, S], BF16, tag=f"ctx{b}", name=f"ctx{b}")
                  for b in range(B)]
        v_sb = [[resid.tile([128, 2, HD + 1], BF16, tag=f"v{b}_{i}",
                            name=f"v{b}_{i}") for i in range(kcs[b])]
                for b in range(B)]

        # whole-batch input staging, one buffer per dj chunk (reused across
        # batches; Tile's WAR deps serialize refill against the last reader)
        xq_st = [xstage.tile([128, S], BF16, tag=f"sxq{i}", name=f"sxq{i}")
                 for i in range(8)]
        xk_st = [xstage.tile([128, S], BF16, tag=f"sxk{i}", name=f"sxk{i}")
                 for i in range(8)]
        xv_st = [xstage.tile([128, S], BF16, tag=f"sxv{i}", name=f"sxv{i}")
                 for i in range(8)]

        def prefetch_kv(b):
            nk = kcs[b] * 128
            for i in range(8):
                nc.gpsimd.dma_start(xk_st[i][:, 0:nk],
                                    ap[f"xk{b}"][i * 128:(i + 1) * 128, :])
            for i in range(8):
                nc.gpsimd.dma_start(xv_st[i][:, 0:nk],
                                    ap[f"xv{b}"][i * 128:(i + 1) * 128, :])

        def prefetch_q(b):
            for i in range(8):
                nc.gpsimd.dma_start(xq_st[i][:],
                                    ap[f"xq{b}"][i * 128:(i + 1) * 128, :])

        def prefetch_s0(b):
            # arrival-ordered 1024-col chunks so stage-0 projections can
            # start as soon as the first column window lands (K, then V on
            # the Pool queue; Q behind the weights on the Sync queue)
            nk = kcs[b] * 128
            for st, apn in ((xk_st, f"xk{b}"), (xv_st, f"xv{b}")):
                cuts = [0, 512, 1024] + list(range(2048, nk + 1, 1024))
                cuts = sorted(set(c for c in cuts if c <= nk) | {nk})
                for n0, n1 in zip(cuts[:-1], cuts[1:]):
                    for i in range(8):
                        nc.gpsimd.dma_start(
                            st[i][:, n0:n1],
                            ap[apn][i * 128:(i + 1) * 128, n0:n1])
            for c in range(2):
                for i in range(8):
                    nc.sync.dma_start(
                        xq_st[i][:, c * 1024:(c + 1) * 1024],
                        ap[f"xq{b}"][i * 128:(i + 1) * 128,
                                     c * 1024:(c + 1) * 1024])

        fill = es.enter_context(
            tc.tile_pool(name="fill_psum", bufs=1, space="PSUM"))
        at_psum = es.enter_context(
            tc.tile_pool(name="at_psum", bufs=1, space="PSUM"))

        # p-state warmup: ~7us of dependency-free matmuls on the already-
        # loaded weight tiles so the tensor engine ramps to full clock
        # while the first input chunks are still in flight
        for w in range(48):
            pw = fill.tile([128, 128], F32,
                           tag=("pqk" if w % 2 == 0 else "pv"), name="pw")
            nc.tensor.matmul(pw[:], wq_sb[w % 8][:], wk_sb[(w + 1) % 8][:],
                             start=True, stop=True)

        state = {"drain": False}

        def ps_copy(dst, src_ap):
            if state["drain"]:
                nc.scalar.activation(dst, src_ap, AF.Copy)
            else:
                nc.vector.tensor_copy(dst, src_ap)

        # ---- filler closures (one PE burst each) ----
        def kproj_block(b, kb):
            nk = kcs[b] * 128
            n = min(512, nk - kb * 512)

            def go():
                psk = fill.tile([128, 512], F32, tag="pqk", name="psk")
                for dj in range(8):
                    nc.tensor.matmul(
                        psk[:, 0:n],
                        wk_sb[dj][:],
                        xk_st[dj][:, kb * 512:kb * 512 + n],
                        start=(dj == 0), stop=(dj == 7))
                ps_copy(kT_sb[b][:, kb * 512:kb * 512 + n], psk[:, 0:n])
            return go

        def vproj_sc(b, sc):
            def go():
                psv = fill.tile([128, 2, HD], F32, tag="pv", name="psv")
                for dj in range(8):
                    nc.tensor.matmul(
                        psv[:], xv_st[dj][:, sc * 128:(sc + 1) * 128],
                        wv_sb[dj][:], start=(dj == 0), stop=(dj == 7))
                nc.vector.tensor_copy(v_sb[b][sc][:, :, 0:HD], psv[:])
                nc.vector.tensor_copy(
                    v_sb[b][sc][:, :, HD], vm_sb[:, b * 16 + sc, :])
            return go

        def qproj_qb(b, qb):
            def go():
                psq = fill.tile([128, 512], F32, tag="pqk", name="psq")
                for dj in range(8):
                    nc.tensor.matmul(
                        psq[:], wq_sb[dj][:],
                        xq_st[dj][:, qb * 512:(qb + 1) * 512],
                        start=(dj == 0), stop=(dj == 7))
                ps_copy(qT_sb[b][:, qb * 512:(qb + 1) * 512], psq[:])
            return go

        def oproj_sc(b, sc):
            def go():
                ot = otp.tile([128, 2, 512], F16, tag="ot", name="ot")
                for ih in range(2):
                    po = fill.tile([128, 512], F32,
                                   tag=("pqk" if ih == 0 else "pv"), name="po")
                    nc.tensor.matmul(
                        po[:], ctx_sb[b][:, sc * 128:(sc + 1) * 128],
                        wo_sb[:, ih * 512:(ih + 1) * 512],
                        start=True, stop=True)
                    if ih == 1:
                        ps_copy(ot[:, ih, :], po[:])
                    else:
                        nc.vector.tensor_copy(ot[:, ih, :], po[:])
                nc.sync.dma_start(
                    ap["out"][b, sc * 128:(sc + 1) * 128, :], ot[:])
            return go

        def proj_closures(b):
            cl = []
            nk = kcs[b] * 128
            for kb in range((nk + 511) // 512):
                cl.append(kproj_block(b, kb))
            for sc in range(kcs[b]):
                cl.append(vproj_sc(b, sc))
            for qb in range(NQB):
                cl.append(qproj_qb(b, qb))
            return cl

        # stage order: descending kc, but smallest second-to-last so the
        # final stage has enough attention iterations to host o-proj fillers
        stages = sorted(range(B), key=lambda b: -kcs[b])
        stages = stages[:-2] + [stages[-1], stages[-2]]

        proj_fifo = deque()
        oproj_fifo = deque()

        def pop_filler():
            if proj_fifo:
                proj_fifo.popleft()()
                return True
            if oproj_fifo:
                oproj_fifo.popleft()()
                return True
            return False

        def attn(b):
            """Attention for batch b; fillers run between sc(kc+1)/av(kc)."""
            kcb = kcs[b]
            it = 0
            for qb in range(NQB):
                # o-proj of group qb-2 is safely complete by now
                if qb >= 2:
                    for sc in range(4 * (qb - 2), 4 * (qb - 1)):
                        oproj_fifo.append(oproj_sc(b, sc))
                av = at_psum.tile([HD + 1, 2, 512], F32, tag="av", name="av")

                def sc_mm(kc):
                    scp = at_psum.tile([128, 1024], F32, tag="sc",
                                       name="scp", bufs=2)
                    for h2 in range(2):
                        nc.tensor.matmul(
                            scp[:, h2 * 512:(h2 + 1) * 512],
                            kT_sb[b][64 * h2:64 * h2 + 64,
                                     kc * 128:(kc + 1) * 128],
                            qT_sb[b][64 * h2:64 * h2 + 64,
                                     qb * 512:(qb + 1) * 512],
                            start=True, stop=True)
                    return scp

                scp_cur = sc_mm(0)
                for kc in range(kcb):
                    scp_next = sc_mm(kc + 1) if kc + 1 < kcb else None
                    ex = expool.tile([128, 1024], BF16, tag="ex", name="ex")
                    nc.scalar.activation(ex[:], scp_cur[:], AF.Exp,
                                         scale=0.125)
                    # paced filler between sc(kc+1) and av(kc): it covers
                    # the exp latency and keeps the PE p-state high
                    if it % 2 == 1 or len(oproj_fifo) > 8:
                        pop_filler()
                    it += 1
                    for h2 in range(2):
                        nc.tensor.matmul(
                            av[:, h2, :], v_sb[b][kc][:, h2, :],
                            ex[:, h2 * 512:(h2 + 1) * 512],
                            start=(kc == 0), stop=(kc == kcb - 1))
                    scp_cur = scp_next

                # normalization: ctx[m, q] = av[m, q] / av[64, q]
                avc = wrk.tile([HD + 1, 2, 512], F32, tag="avc", name="avc")
                nc.vector.tensor_copy(avc[:], av[:])
                r0 = wrk.tile([1, 1024], F32, tag="r0", name="r0")
                nc.sync.dma_start(r0[:], avc[HD:HD + 1, :, :])
                bc = wrk.tile([HD, 1024], F32, tag="bc", name="bc")
                nc.gpsimd.partition_broadcast(bc[:], r0[0:1, :])
                recb = wrk.tile([HD, 1024], F32, tag="recb", name="recb")
                nc.vector.reciprocal_approx_fast(recb[:], bc[:])
                nc.vector.tensor_mul(
                    ctx_sb[b][0:HD, qb * 512:(qb + 1) * 512],
                    avc[0:HD, 0, :], recb[:, 0:512])
                tmp = wrk.tile([HD, 512], BF16, tag="tmpb", name="tmp")
                nc.vector.tensor_mul(tmp[:], avc[0:HD, 1, :],
                                     recb[:, 512:1024])
                nc.sync.dma_start(
                    ctx_sb[b][HD:128, qb * 512:(qb + 1) * 512], tmp[:])
            # last two qb groups become available for later stages
            for sc in range(4 * (NQB - 2), 4 * NQB):
                oproj_fifo.append(oproj_sc(b, sc))

        # ---- stage 0: inline projection, then pipelined attention ----
        s0 = stages[0]
        prefetch_s0(s0)
        for cl in proj_closures(s0):
            cl()
        if len(stages) > 1:
            prefetch_kv(stages[1])
            prefetch_q(stages[1])

        for si, b in enumerate(stages):
            if si + 1 < len(stages):
                proj_fifo.extend(proj_closures(stages[si + 1]))
                if si + 2 < len(stages):
                    nb = stages[si + 2]
                    proj_fifo.append(lambda nb=nb: prefetch_kv(nb))
                    proj_fifo.append(lambda nb=nb: prefetch_q(nb))
            attn(b)
            # drain projections for the next stage before its attention;
            # PSUM copies go to ACT here (no exp work competing)
            state["drain"] = True
            while proj_fifo:
                proj_fifo.popleft()()
            state["drain"] = False
        state["drain"] = True
        while oproj_fifo:
            oproj_fifo.popleft()()
        state["drain"] = False


def _build(kcs):
    key = ("nc", tuple(kcs))
    if key in _cache:
        return _cache[key]
    nc = bacc.Bacc("TRN2", target_bir_lowering=False, debug=False, num_devices=8)
    ap = {"wq": nc.dram_tensor("wq", [D, 128], BF16, kind="ExternalInput").ap(),
          "wk": nc.dram_tensor("wk", [D, 128], BF16, kind="ExternalInput").ap(),
          "wv": nc.dram_tensor("wv", [D, 2, HD], BF16, kind="ExternalInput").ap(),
          "wo": nc.dram_tensor("wo", [128, D], BF16, kind="ExternalInput").ap(),
          "vones": nc.dram_tensor("vones", [128, 64, 2], BF16,
                                  kind="ExternalInput").ap(),
          "out": nc.dram_tensor("out", [B, S, D], F16,
                                kind="ExternalOutput").ap()}
    for b in range(B):
        ap[f"xq{b}"] = nc.dram_tensor(f"xq{b}", [D, S], BF16,
                                      kind="ExternalInput").ap()
        ap[f"xk{b}"] = nc.dram_tensor(f"xk{b}", [D, kcs[b] * 128], BF16,
                                      kind="ExternalInput").ap()
        ap[f"xv{b}"] = nc.dram_tensor(f"xv{b}", [D, kcs[b] * 128], BF16,
                                      kind="ExternalInput").ap()
    with tile.TileContext(nc) as tc:
        _emit(nc, tc, ap, kcs)
    nc.compile()
    _cache[key] = nc
    return nc


def _in_maps(kcs, queries, keys, values, valid_len, W_q, W_k, W_v, W_o):
    bf = ml_dtypes.bfloat16
    # host-masked ones column: 1 where k < valid_len[b], else 0
    # vones[p, b*16+sc, h] = 1 if sc*128+p < valid_len[b] else 0
    kpos = np.arange(16 * 128).reshape(16, 128)
    vones = np.zeros((128, 64, 2), bf)
    for b in range(B):
        v1 = (kpos < int(valid_len[b])).astype(bf)  # [16, 128]
        vones[:, b * 16:(b + 1) * 16, :] = v1.T[:, :, None]
    maps = []
    for c in range(8):
        j0 = 128 * c
        m = {
            "wq": np.ascontiguousarray(W_q[j0:j0 + 128, :].T).astype(bf),
            "wk": np.ascontiguousarray(W_k[j0:j0 + 128, :].T).astype(bf),
            "wv": np.ascontiguousarray(
                W_v[j0:j0 + 128, :].T).astype(bf).reshape(D, 2, HD),
            "wo": np.ascontiguousarray(W_o[:, j0:j0 + 128].T).astype(bf),
            "vones": vones,
        }
        for b in range(B):
            nk = kcs[b] * 128
            xv = values[b][:nk].T.copy()      # [D, nk]
            xv[:, int(valid_len[b]):] = 0.0   # mask padding rows of v
            m[f"xq{b}"] = np.ascontiguousarray(queries[b].T).astype(bf)
            m[f"xk{b}"] = np.ascontiguousarray(keys[b][:nk].T).astype(bf)
            m[f"xv{b}"] = xv.astype(bf)
        maps.append(m)
    return maps


def kernel(queries, keys, values, valid_len, W_q, W_k, W_v, W_o, _run_kwargs=None):
    queries = np.asarray(queries, np.float32)
    keys = np.asarray(keys, np.float32)
    values = np.asarray(values, np.float32)
    valid_len = np.asarray(valid_len)
    W_q = np.asarray(W_q, np.float32)
    W_k = np.asarray(W_k, np.float32)
    W_v = np.asarray(W_v, np.float32)
    W_o = np.asarray(W_o, np.float32)

    kcs = [max(1, min(16, -(-int(valid_len[b]) // 128))) for b in range(B)]
    nc = _build(kcs)
    maps = _in_maps(kcs, queries, keys, values, valid_len, W_q, W_k, W_v, W_o)
    res = run_bass_kernel_spmd(nc, maps, list(range(8)), **(_run_kwargs or {}))
    out = np.zeros((B, S, D), np.float32)
    for c in range(8):
        out += res.results[c]["out"].astype(np.float32)
    if _run_kwargs:
        _cache["last_results"] = res
    return out
